# revision 12
# baseline (speedup 1.0000x reference)
"""HGT layer (heterogeneous graph transformer) on 8 Trainium2 NeuronCores.

v2: dst-partitioned as v1, but device pipeline restructured around groups of
G=4 edge-blocks (512 edges) to amortize per-instruction overhead:
  - PE: per block rec=hs^T@Wkv (N=256), qx=At^T@Q (N=128), agg+=A^T@msg
    (N=132); groups write into wide PSUM tiles (rec4 2 banks, qx4 1 bank).
  - DVE (3 big ops/group): prod4 = rec_k * qx -> bf16; score reduce
    (per-head); msg4 = rec_v * esc (esc broadcast via stride-0 AP).
  - GpSimd: one-hot A4 build (iota==dst lane) + esc->msg z-column copy.
  - ACT: exp(scores), Q/T PSUM->SBUF copies.
  - Groups are stream-aligned (may straddle dst tiles); Q and agg tiles are
    created lazily per tile.
Host prep as v1 (per-edge pre-gathered source features, per-dst-tile grouped
padded blocks), plus: dst lanes stored bf16, stream padded to multiple of 4.
Weight folding identical to v1.
"""

import math
import os

import numpy as np
import ml_dtypes

BF16 = ml_dtypes.bfloat16

NPAP, NAUT = 100000, 50000
D, H, DK = 128, 4, 32
NCORES = 8
PPC, APC = NPAP // NCORES, NAUT // NCORES  # 12500, 6250
PT = (PPC + 127) // 128  # 98 paper tiles / core
AT = (APC + 127) // 128  # 49 author tiles / core
G = 4    # blocks per group
GH = 8   # hsT blocks per DMA group
GD = 256  # dst blocks per DMA group

LAST_RESULT = {}


def _prep_relation(src, dst, h_src_ext, n_per_core, ntiles):
    """Partition edges by dst owner core, group by dst tile, pad to uniform
    budgets; pad stream length to a multiple of G. Returns (nblk budgets,
    NB, per-core hsT [NB,128,128] bf16, per-core A [NB,128(edge),128(dst)]
    bf16 one-hot, per-core A^T blocks [NB,128,128] bf16)."""
    core = dst // n_per_core
    dloc = dst - core * n_per_core
    tl = dloc >> 7
    lane = (dloc & 127).astype(np.float32)

    cnt = np.bincount(core * ntiles + tl, minlength=NCORES * ntiles).reshape(
        NCORES, ntiles
    )
    nblk = (cnt.max(axis=0) + 127) // 128  # blocks per tile (uniform)
    pad = (-int(nblk.sum())) % G
    nblk[-1] += pad  # trailing pad blocks live in the last tile
    NB = int(nblk.sum())
    tile_slot0 = np.concatenate([[0], np.cumsum(nblk)]) * 128

    hsT_cores, dstT_cores, at_cores = [], [], []
    zero_row = h_src_ext.shape[0] - 1  # h_src_ext has appended zero row
    for c in range(NCORES):
        sel = np.nonzero(core == c)[0]
        tl_c = tl[sel]
        order = np.argsort(tl_c, kind="stable")
        sel_o = sel[order]
        tl_s = tl_c[order]
        start_of = np.searchsorted(tl_s, np.arange(ntiles))
        within = np.arange(len(sel_o)) - start_of[tl_s]
        slot = tile_slot0[tl_s] + within

        src_slots = np.full(NB * 128, zero_row, np.int64)
        src_slots[slot] = src[sel_o]
        lane_slots = np.full(NB * 128, 255.0, np.float32)
        lane_slots[slot] = lane[sel_o]

        mat = h_src_ext[src_slots]  # [NB*128, 128] f32
        # packed streams [128, NB*128]: per-partition contiguous for DMA
        hsT = np.ascontiguousarray(mat.T).astype(BF16)
        # A[b, e, d] one-hot -> stream [e, b*128+d]
        ab = (
            lane_slots.reshape(NB, 128, 1)
            == np.arange(128, dtype=np.float32)[None, None, :]
        ).astype(BF16)
        ab = np.ascontiguousarray(ab.transpose(1, 0, 2).reshape(128, NB * 128))
        # A^T[b, d, e] one-hot -> stream [d, b*128+e]
        at = (
            np.arange(128, dtype=np.float32)[None, :, None]
            == lane_slots.reshape(NB, 1, 128)
        ).astype(BF16)
        at = np.ascontiguousarray(at.transpose(1, 0, 2).reshape(128, NB * 128))
        hsT_cores.append(hsT)
        dstT_cores.append(ab)
        at_cores.append(at)
    return nblk, NB, hsT_cores, dstT_cores, at_cores


def _prep_dst_type(h, n_per_core, ntiles):
    hdT, hrow = [], []
    for c in range(NCORES):
        rows = h[c * n_per_core : (c + 1) * n_per_core]
        pad = np.zeros((ntiles * 128, D), np.float32)
        pad[: rows.shape[0]] = rows
        t = pad.reshape(ntiles, 128, D)
        hdT.append(np.ascontiguousarray(t.transpose(0, 2, 1)).astype(BF16))
        hrow.append(np.ascontiguousarray(t))
    return hdT, hrow


def _fold_weights(Wk, Wv, Wq, Wa, rel_att, rel_msg, rel_pri, skip):
    sqrt_dk = math.sqrt(DK)
    rel_ts = [0, 1, 0]  # cites: paper, writes: author, rev: paper
    wkv = []
    for e in range(3):
        ts = rel_ts[e]
        ratt = rel_att[e] * (rel_pri[e][:, None, None] / sqrt_dk)
        watt = np.einsum(
            "hiI,hij->Ihj", Wk[ts].reshape(H, DK, D), ratt
        ).reshape(D, D)
        wmsg = np.einsum(
            "hiI,hij->Ihj", Wv[ts].reshape(H, DK, D), rel_msg[e]
        ).reshape(D, D)
        wkv.append(np.ascontiguousarray(np.concatenate([watt, wmsg], 1)).astype(BF16))
    wq = [np.ascontiguousarray(Wq[t].T).astype(BF16) for t in range(2)]
    alpha = 1.0 / (1.0 + np.exp(-skip.astype(np.float64)))
    waT = [
        np.ascontiguousarray(Wa[0].T * alpha[0] * 0.5).astype(BF16),
        np.ascontiguousarray(Wa[1].T * alpha[1]).astype(BF16),
    ]
    return wkv, wq, waT, alpha


def kernel(**inputs):
    from concourse import bacc, bass, mybir, tile
    from concourse.bass import broadcast_tensor_aps
    from concourse.bass_utils import run_bass_kernel_spmd

    inp = {k: np.asarray(v) for k, v in inputs.items()}
    h_paper = inp["h_paper"].astype(np.float32)
    h_author = inp["h_author"].astype(np.float32)
    for bname in ("bk", "bq", "bv", "ba"):
        assert not np.any(inp[bname]), f"nonzero bias {bname} unsupported"

    wkv, wq, waT, alpha = _fold_weights(
        inp["Wk"].astype(np.float32), inp["Wv"].astype(np.float32),
        inp["Wq"].astype(np.float32), inp["Wa"].astype(np.float32),
        inp["rel_att"].astype(np.float32), inp["rel_msg"].astype(np.float32),
        inp["rel_pri"].astype(np.float32), inp["skip"].astype(np.float32),
    )

    hp_ext = np.concatenate([h_paper, np.zeros((1, D), np.float32)], 0)
    ha_ext = np.concatenate([h_author, np.zeros((1, D), np.float32)], 0)

    nblk_c, NBC, hsT_c, ab_c, at_c = _prep_relation(
        inp["cites_src"].astype(np.int64), inp["cites_dst"].astype(np.int64),
        hp_ext, PPC, PT)
    nblk_w, NBW, hsT_w, ab_w, at_w = _prep_relation(
        inp["writes_src"].astype(np.int64), inp["writes_dst"].astype(np.int64),
        ha_ext, PPC, PT)
    nblk_r, NBR, hsT_r, ab_r, at_r = _prep_relation(
        inp["rev_src"].astype(np.int64), inp["rev_dst"].astype(np.int64),
        hp_ext, APC, AT)

    hdT_p, hrow_p = _prep_dst_type(h_paper, PPC, PT)
    hdT_a, hrow_a = _prep_dst_type(h_author, APC, AT)

    # ---------------- build the SPMD Bass program ----------------
    nc = bacc.Bacc("TRN2", target_bir_lowering=False, debug=False,
                   num_devices=NCORES)
    dt = mybir.dt

    d_hsT, d_ab, d_at = {}, {}, {}
    for nm, nb in (("cites", NBC), ("writes", NBW), ("rev", NBR)):
        d_hsT[nm] = nc.dram_tensor(f"hsT_{nm}", [128, max(nb, 1) * 128],
                                   dt.bfloat16, kind="ExternalInput")
        d_ab[nm] = nc.dram_tensor(f"ab_{nm}", [128, max(nb, 1) * 128],
                                  dt.bfloat16, kind="ExternalInput")
        d_at[nm] = nc.dram_tensor(f"at_{nm}", [128, max(nb, 1) * 128],
                                  dt.bfloat16, kind="ExternalInput")
    d_hdT = {
        0: nc.dram_tensor("hdT_paper", [PT, 128, 128], dt.bfloat16,
                          kind="ExternalInput"),
        1: nc.dram_tensor("hdT_author", [AT, 128, 128], dt.bfloat16,
                          kind="ExternalInput"),
    }
    d_hrow = {
        0: nc.dram_tensor("hrow_paper", [PT, 128, 128], dt.float32,
                          kind="ExternalInput"),
        1: nc.dram_tensor("hrow_author", [AT, 128, 128], dt.float32,
                          kind="ExternalInput"),
    }
    NOUT = (PT + AT) * 128
    d_out = nc.dram_tensor("out", [NOUT, 128], dt.float32, kind="ExternalOutput")

    d_wkv = [nc.inline_tensor(wkv[e], name=f"wkv{e}") for e in range(3)]
    d_wq = [nc.inline_tensor(wq[t], name=f"wq{t}") for t in range(2)]
    d_waT = [nc.inline_tensor(waT[t], name=f"waT{t}") for t in range(2)]
    d_ident = nc.inline_tensor(np.eye(128, dtype=np.float32).astype(BF16),
                               name="identc")

    # rel name -> (dram hsT, dram A, dram A_T, budgets, wkv idx, dst type)
    rel_info = {
        "cites": (d_hsT["cites"], d_ab["cites"], d_at["cites"], nblk_c, 0, 0),
        "writes": (d_hsT["writes"], d_ab["writes"], d_at["writes"], nblk_w,
                   1, 0),
        "rev": (d_hsT["rev"], d_ab["rev"], d_at["rev"], nblk_r, 2, 1),
    }
    # papers aggregate 2 relations into one [128, 2, 132] psum bank
    rel_slot = {"cites": 0, "writes": 1, "rev": 0}

    with tile.TileContext(nc) as tc:
        with (
            tc.tile_pool(name="const", bufs=1) as cpool,
            tc.tile_pool(name="hs", bufs=4) as hs_pool,
            tc.tile_pool(name="qsb", bufs=3) as qsb_pool,
            tc.tile_pool(name="work", bufs=3) as wpool,
            tc.tile_pool(name="tilew", bufs=3) as tpool,
            tc.tile_pool(name="rec_ps", bufs=2, space="PSUM") as rec_ps,
            tc.tile_pool(name="qx_ps", bufs=2, space="PSUM") as qx_ps,
            tc.tile_pool(name="agg_ps", bufs=2, space="PSUM") as agg_ps,
        ):
            # constants to SBUF
            s_wkv = []
            for e in range(3):
                w = cpool.tile([128, 256], dt.bfloat16, name=f"s_wkv{e}")
                nc.sync.dma_start(out=w[:], in_=d_wkv[e][:])
                s_wkv.append(w)
            s_wq, s_waT = [], []
            for t in range(2):
                a = cpool.tile([128, 128], dt.bfloat16, name=f"s_wq{t}")
                nc.sync.dma_start(out=a[:], in_=d_wq[t][:])
                s_wq.append(a)
                b = cpool.tile([128, 128], dt.bfloat16, name=f"s_waT{t}")
                nc.sync.dma_start(out=b[:], in_=d_waT[t][:])
                s_waT.append(b)
            s_ident = cpool.tile([128, 128], dt.bfloat16, name="s_ident")
            nc.sync.dma_start(out=s_ident[:], in_=d_ident[:])

            # streaming state per relation
            gstate = {r: {"g": 0, "hs": None, "at": None, "ab": None}
                      for r in rel_info}

            def get_block(rname):
                st = gstate[rname]
                d_hs, d_a, d_att, _, _, _ = rel_info[rname]
                g = st["g"]
                hi, ho = divmod(g, GH)
                if ho == 0:
                    nb = d_hs.shape[1] // 128
                    n = min(GH, nb - hi * GH)
                    for key, tag, srct in (("hs", "hsg", d_hs),
                                           ("at", "atg", d_att),
                                           ("ab", "abg", d_a)):
                        t = hs_pool.tile([128, GH, 128], dt.bfloat16,
                                         name=tag, tag=tag)
                        nc.sync.dma_start(
                            out=t[:, :n, :],
                            in_=srct[:, hi * GH * 128 : (hi * GH + n) * 128
                                     ].rearrange("p (b c) -> p b c", c=128),
                        )
                        st[key] = t
                st["g"] = g + 1
                return (st["hs"][:, ho, :], st["at"][:, ho, :],
                        st["ab"][:, ho, :])

            # lazy per-tile state
            q_cache = {}     # (ttype, ti) -> SBUF bf16 Q tile
            agg_cache = {}   # (ttype, ti) -> PSUM [128, 2, 132] tile

            def q_of(ttype, ti):
                key = (ttype, ti)
                if key not in q_cache:
                    hdt = tpool.tile([128, 128], dt.bfloat16, name="hdt",
                                     tag="hdt")
                    nc.sync.dma_start(out=hdt[:], in_=d_hdT[ttype][ti, :, :])
                    q_ps = qx_ps.tile([128, 4, 128], dt.float32, name="q_ps",
                                      tag="qx4")
                    nc.tensor.matmul(q_ps[:, 0, :], lhsT=hdt[:],
                                     rhs=s_wq[ttype][:], start=True, stop=True)
                    Q = qsb_pool.tile([128, 128], dt.bfloat16, name="Q",
                                      tag="Q")
                    nc.scalar.copy(out=Q[:], in_=q_ps[:, 0, :])
                    q_cache[key] = Q
                return q_cache[key]

            def agg_of(ttype, ti):
                key = (ttype, ti)
                if key not in agg_cache:
                    agg_cache[key] = agg_ps.tile([128, 2, 132], dt.float32,
                                                 name="agg", tag="agg")
                return agg_cache[key]

            def emit_group(rname, binfo):
                """binfo: list of (tile_idx, within_idx, nblk_of_tile) for the
                g blocks (g<=G) of this group."""
                _, _, _, _, widx, ttype = rel_info[rname]
                g = len(binfo)
                hs_l, at_l, ab_l = [], [], []
                for k in range(g):
                    hs_b, at_b, ab_b = get_block(rname)
                    hs_l.append(hs_b)
                    at_l.append(at_b)
                    ab_l.append(ab_b)

                rec4 = rec_ps.tile([128, G, 256], dt.float32, name="rec4",
                                   tag="rec4")
                qx4 = qx_ps.tile([128, 4, 128], dt.float32, name="qx4",
                                 tag="qx4")
                for k in range(g):
                    nc.tensor.matmul(rec4[:, k, :], lhsT=hs_l[k],
                                     rhs=s_wkv[widx][:], start=True, stop=True)
                    nc.tensor.matmul(qx4[:, k, :], lhsT=at_l[k],
                                     rhs=q_of(ttype, binfo[k][0])[:],
                                     start=True, stop=True)

                # qx -> SBUF (DVE cannot take two PSUM operands)
                qxs = wpool.tile([128, G, 128], dt.bfloat16, name="qxs",
                                 tag="qxs")
                nc.scalar.copy(out=qxs[:, :g, :], in_=qx4[:, :g, :])

                # prod = k * qx -> bf16
                prodb = wpool.tile([128, G, 4, 32], dt.bfloat16, name="prodb",
                                   tag="prodb")
                nc.vector.tensor_tensor(
                    out=prodb[:, :g, :, :],
                    in0=rec4[:, :g, 0:128].rearrange("p b (h i) -> p b h i",
                                                     h=4),
                    in1=qxs[:, :g, :].rearrange("p b (h i) -> p b h i", h=4),
                    op=mybir.AluOpType.mult)
                # per-head score sums
                scores = wpool.tile([128, G, 4], dt.float32, name="scores",
                                    tag="scores")
                nc.vector.tensor_reduce(
                    out=scores[:, :g, :], in_=prodb[:, :g, :, :],
                    axis=mybir.AxisListType.X, op=mybir.AluOpType.add)
                esc = wpool.tile([128, G, 4, 1], dt.bfloat16, name="esc",
                                 tag="esc")
                nc.scalar.activation(
                    out=esc[:, :g, :, 0], in_=scores[:, :g, :],
                    func=mybir.ActivationFunctionType.Exp)

                # msg = [v * esc | esc]
                msg4 = wpool.tile([128, G, 132], dt.bfloat16, name="msg4",
                                  tag="msg4")
                m0, m1 = broadcast_tensor_aps(
                    rec4[:, :g, 128:256].rearrange("p b (h i) -> p b h i",
                                                   h=4),
                    esc[:, :g, :, :])
                nc.vector.tensor_tensor(
                    out=msg4[:, :g, 0:128].rearrange("p b (h i) -> p b h i",
                                                     h=4),
                    in0=m0, in1=m1, op=mybir.AluOpType.mult)
                nc.vector.tensor_copy(out=msg4[:, :g, 128:132],
                                       in_=esc[:, :g, :, 0])

                for k in range(g):
                    ti, wi, nb_t = binfo[k]
                    agg = agg_of(ttype, ti)
                    nc.tensor.matmul(agg[:, rel_slot[rname], :],
                                     lhsT=ab_l[k],
                                     rhs=msg4[:, k, :],
                                     start=(wi == 0), stop=(wi == nb_t - 1))

            def finalize(ttype, ti, have):
                """have: list of rel names with >=1 block in this tile."""
                agg2 = agg_cache.pop((ttype, ti), None)
                if agg2 is None:
                    have = []
                orow = ti * 128 if ttype == 0 else (PT + ti) * 128
                out_s = tpool.tile([128, 128], dt.float32, name="out_s",
                                   tag="out_s")
                hrow = tpool.tile([128, 128], dt.float32, name="hrow",
                                  tag="hrow")
                nc.sync.dma_start(out=hrow[:], in_=d_hrow[ttype][ti, :, :])
                nr = len(have)
                if nr:
                    zb = wpool.tile([128, 2, 4], dt.float32, name="zb",
                                    tag="zb")
                    nc.vector.tensor_scalar(
                        out=zb[:, :nr, :], in0=agg2[:, :nr, 128:132],
                        scalar1=1e-30, scalar2=None,
                        op0=mybir.AluOpType.add)
                    rz = wpool.tile([128, 2, 4, 1], dt.float32, name="rz",
                                    tag="rz")
                    nc.vector.reciprocal(out=rz[:, :nr, :, 0],
                                         in_=zb[:, :nr, :])
                    Ts = []
                    for s in range(nr):
                        T = tpool.tile([128, 128], dt.bfloat16, name=f"T{s}",
                                       tag=f"T{s}")
                        t0, t1 = broadcast_tensor_aps(
                            agg2[:, s, 0:128].rearrange("p (h i) -> p h i",
                                                        h=4),
                            rz[:, s, :, :])
                        nc.vector.tensor_tensor(
                            out=T[:].rearrange("p (h i) -> p h i", h=4),
                            in0=t0, in1=t1, op=mybir.AluOpType.mult)
                        Ts.append(T)
                    Tc = Ts[0]
                    if nr == 2:
                        Tsum = tpool.tile([128, 128], dt.bfloat16,
                                          name="Tsum", tag="Tsum")
                        nc.vector.tensor_tensor(out=Tsum[:], in0=Ts[0][:],
                                                in1=Ts[1][:],
                                                op=mybir.AluOpType.add)
                        Tc = Tsum
                    tt_ps = qx_ps.tile([128, 4, 128], dt.bfloat16,
                                       name="tt_ps", tag="qx4")
                    nc.tensor.transpose(tt_ps[:, 0, :], Tc[:], s_ident[:])
                    Tt = tpool.tile([128, 128], dt.bfloat16, name="Tt",
                                    tag="Tt")
                    nc.scalar.copy(out=Tt[:], in_=tt_ps[:, 0, :])
                    out_ps = qx_ps.tile([128, 4, 128], dt.float32,
                                        name="out_ps", tag="qx4")
                    nc.tensor.matmul(out_ps[:, 0, :], lhsT=Tt[:],
                                     rhs=s_waT[ttype][:], start=True,
                                     stop=True)
                    nc.vector.scalar_tensor_tensor(
                        out=out_s[:], in0=hrow[:],
                        scalar=float(1.0 - alpha[ttype]), in1=out_ps[:, 0, :],
                        op0=mybir.AluOpType.mult, op1=mybir.AluOpType.add)
                else:
                    nc.vector.tensor_scalar(
                        out=out_s[:], in0=hrow[:],
                        scalar1=float(1.0 - alpha[ttype]), scalar2=None,
                        op0=mybir.AluOpType.mult)
                nc.sync.dma_start(out=d_out[orow : orow + 128, :], in_=out_s[:])

            # ---- host-side stream metadata ----
            def stream_plan(nblk):
                tiles = np.repeat(np.arange(len(nblk)), nblk)
                within = np.concatenate([np.arange(n) for n in nblk]) \
                    if len(nblk) else np.array([], np.int64)
                return tiles.astype(int), within.astype(int)

            plans = {r: stream_plan(rel_info[r][3]) for r in rel_info}
            cursors = {r: 0 for r in rel_info}

            def pump(rname, upto_tile):
                """Emit whole groups until stream covers all blocks of tiles
                <= upto_tile."""
                tiles, within = plans[rname]
                nblk = rel_info[rname][3]
                total = len(tiles)
                end = int(np.searchsorted(tiles, upto_tile, side="right"))
                c = cursors[rname]
                while c < end:
                    g = min(G, total - c)
                    binfo = [(int(tiles[c + k]), int(within[c + k]),
                              int(nblk[tiles[c + k]])) for k in range(g)]
                    emit_group(rname, binfo)
                    c += g
                cursors[rname] = c

            for ti in range(PT):
                pump("cites", ti)
                pump("writes", ti)
                have = [r for r in ("cites", "writes")
                        if rel_info[r][3][ti] > 0]
                finalize(0, ti, have)
            for ti in range(AT):
                pump("rev", ti)
                have = ["rev"] if nblk_r[ti] > 0 else []
                finalize(1, ti, have)

    nc.compile()

    if os.environ.get("HGT_BUILD_ONLY"):
        return np.zeros((NPAP + NAUT, D), np.float32)

    in_maps = []
    for c in range(NCORES):
        in_maps.append({
            "hsT_cites": hsT_c[c], "hsT_writes": hsT_w[c], "hsT_rev": hsT_r[c],
            "ab_cites": ab_c[c], "ab_writes": ab_w[c], "ab_rev": ab_r[c],
            "at_cites": at_c[c], "at_writes": at_w[c], "at_rev": at_r[c],
            "hdT_paper": hdT_p[c], "hdT_author": hdT_a[c],
            "hrow_paper": hrow_p[c], "hrow_author": hrow_a[c],
        })

    trace = bool(int(os.environ.get("HGT_TRACE", "0")))
    res = run_bass_kernel_spmd(nc, in_maps, list(range(NCORES)), trace=trace)
    LAST_RESULT["exec_time_ns"] = res.exec_time_ns
    LAST_RESULT["res"] = res
    LAST_RESULT["nc"] = nc
    LAST_RESULT["in_maps"] = in_maps

    out = np.empty((NPAP + NAUT, D), np.float32)
    for c in range(NCORES):
        o = np.asarray(res.results[c]["out"], np.float32)
        out[c * PPC : (c + 1) * PPC] = o[:PPC]
        out[NPAP + c * APC : NPAP + (c + 1) * APC] = o[PT * 128 : PT * 128 + APC]
    return out
